# revision 1
# baseline (speedup 1.0000x reference)
"""Causal attention kernel for Trainium2 (Bass/Tile), data-parallel over batch.

Problem (hardcoded): x[64,512,1024] f32, Wq/Wk/Wv[1024,256], bq/bk/bv[256].
  q = x@Wq+bq ; k = x@Wk+bk ; v = x@Wv+bv
  out = softmax(causal(q k^T / sqrt(256))) @ v           -> [64,512,256]

Sharding: 8 NeuronCores, 8 batches per core (pure data parallel, weights
replicated, no collectives). Each core runs the same program on its shard.

Per-core pipeline (batches processed in pairs):
  1. DMA x[b] per 128-token chunk -> SBUF; PE-transpose (fp32r, grouped 4 per
     PSUM bank, one wide DVE drain) -> xT [128(dm), 8, 512].
  2. qT/kT = W.T @ x.T via fp32r matmuls (head dim on partitions, one weight
     load feeding both batches of the pair); bias + 1/sqrt(d) scaling folded
     into the ACT-engine PSUM->SBUF copy-back. v in natural layout
     [tk, d] (lhsT = xT chunk); its bias is folded through the softmax and
     added to the output instead (softmax rows sum to 1).
  3. Per 128-row query chunk c (software-pipelined S/T/V stages so the PE
     always has fill work): scores psum over keys [0,(c+1)*128); additive
     causal mask on the diagonal block only; single Exp (no max-subtraction
     -- scores are O(1)) that also emits the row-sum via accum_out.
  4. PE-transpose the exp'd weights (fp32r), AV matmul, 1/rowsum scaling on
     ACT, +bv on GPSIMD, per-chunk DMA out on the GPSIMD queues.

All matmuls run as float32r (TF32-like, 1 cycle/row at free-dim >= 256 vs 4
for fp32): ~2e-4 relative error vs the fp32 reference.
"""

import numpy as np

import concourse.bass as bass
import concourse.mybir as mybir
import concourse.tile as tile
from concourse import bacc
from concourse.bass_utils import run_bass_kernel_spmd
from concourse.masks import make_causal_mask, make_identity

B, T, DM, D = 64, 512, 1024, 256
NCORES = 8
BPC = B // NCORES  # batches per core
P = 128
KO = DM // P  # 8 contraction subtiles for the projections
NCH = T // P  # 4 token chunks per sequence
DJ = D // P  # 2 head-dim chunks
SCALE = 1.0 / 16.0  # 256 ** -0.5
MASK_VAL = -1e30

F32 = mybir.dt.float32
F32R = mybir.dt.float32r


def emit_core_program(ctx, nc: bass.Bass, tc, io, reps=1, hints=True,
                      split_x=True, stv=True, pair_qk=True, gp_store=True, dual=True,
                      alt_drain=False, staggered=False, xpair=False, c0pad=True,
                      xq_split=True, vt_proj=False):
    x_d, wq_d, bq_d, wk_d, bk_d, wv_d, bv_d, out_d = io
    X = mybir.AxisListType.X

    def enter_pool(name, bufs, space="SBUF"):
        return ctx.enter_context(tc.tile_pool(name=name, bufs=bufs, space=space))

    consts = enter_pool("consts", bufs=1)
    ident = consts.tile([P, P], F32, name="ident")
    make_identity(nc, ident)
    identr = consts.tile([P, P], F32R, name="identr")
    nc.vector.tensor_copy(identr, ident)
    cmask = consts.tile([P, P], F32, name="cmask")
    make_causal_mask(nc, cmask, mask_val=MASK_VAL)
    cfull = consts.tile([P, P], F32, name="cfull")
    nc.gpsimd.memset(cfull, MASK_VAL)

    wq_s = consts.tile([P, KO, D], F32R, name="wq_s")
    wk_s = consts.tile([P, KO, D], F32R, name="wk_s")
    wv_s = consts.tile([P, KO, D], F32R, name="wv_s")
    bq_s = consts.tile([P, DJ], F32, name="bq_s")
    bk_s = consts.tile([P, DJ], F32, name="bk_s")
    bq16_s = consts.tile([P, DJ], F32, name="bq16_s")
    bv_s = consts.tile([P, D], F32, name="bv_s")

    def load_consts():
        # issued after the first x-chunk DMAs so the transposes start early;
        # weights ride the ACT hardware queue, biases the gpsimd queues
        nc.scalar.dma_start(wq_s, wq_d.rearrange("(ko p) d -> p ko d", p=P).bitcast(F32R))
        nc.scalar.dma_start(wk_s, wk_d.rearrange("(ko p) d -> p ko d", p=P).bitcast(F32R))
        nc.scalar.dma_start(wv_s, wv_d.rearrange("(ko p) d -> p ko d", p=P).bitcast(F32R))
        nc.gpsimd.dma_start(bq_s, bq_d.rearrange("(j p) -> p j", p=P))
        nc.gpsimd.dma_start(bk_s, bk_d.rearrange("(j p) -> p j", p=P))
        nc.vector.tensor_scalar_mul(bq16_s, bq_s, SCALE)
        nc.gpsimd.dma_start(bv_s, bv_d[None, :].to_broadcast((P, D)))

    x_pool = enter_pool("x", bufs=3)
    xt_pool = enter_pool("xt", bufs=3)
    qkv_pool = enter_pool("qkv", bufs=2)
    w_pool = enter_pool("w", bufs=4 if dual else 3)
    wt_pool = enter_pool("wt", bufs=4 if dual else 2)
    o_pool = enter_pool("o", bufs=2)
    stat_pool = enter_pool("stat", bufs=8)
    ps_tr = enter_pool("ps_tr", bufs=2, space="PSUM")
    ps_mm = enter_pool("ps_mm", bufs=2, space="PSUM")
    ps_s = enter_pool("ps_s", bufs=2, space="PSUM")
    ps_av = enter_pool("ps_av", bufs=2, space="PSUM")

    if reps > 1:
        he = (
            mybir.EngineType.PE, mybir.EngineType.DVE,
            mybir.EngineType.Activation, mybir.EngineType.SP,
        ) if hints else ()
        ctx.enter_context(tc.For_i(0, reps, 1, hint_engines=he,
                                   staggered_reset=staggered))

    def load_stages(b):
        """Cross-pair pipelined form: returns (xt, [emit-closures])."""
        x_sb = x_pool.tile([P, NCH, DM], F32R, name="x_sb", tag="x_sb")
        xt = xt_pool.tile([P, KO, T], F32R, name="xt", tag="xt")
        xr = x_d[b].rearrange("(c p) m -> p c m", p=P).bitcast(F32R)

        def dma_stage():
            for c in range(NCH):
                eng = nc.scalar if (xq_split and c % 2) else nc.sync
                eng.dma_start(x_sb[:, c, :], xr[:, c, :])

        def tr_stage(ko):
            pt = ps_tr.tile([P, NCH, P], F32R, name="pt", tag="pt")
            for c in range(NCH):
                nc.tensor.transpose(
                    pt[:, c, :], x_sb[:, c, ko * P:(ko + 1) * P], identr
                )
            nc.vector.tensor_copy(xt[:, ko, :], pt)

        return xt, [dma_stage] + [
            (lambda ko=ko: tr_stage(ko)) for ko in range(KO)
        ]

    def qk_proj_stages(xts):
        """Returns ((qts, kts), [emit-closures]) -- one closure per (proj, j)
        group of 16 paired matmuls + 2 ACT drains."""
        dsts = {}
        for lbl in ("q", "k"):
            dsts[lbl] = [
                qkv_pool.tile([P, DJ, T], F32R, name="qkt", tag=f"qkt{i}{lbl}")
                for i in range(len(xts))
            ]

        def group(lbl, w_s, b_s, scl, j):
            pms = [ps_mm.tile([P, T], F32, name="pm", tag="pm") for _ in xts]
            for ko in range(KO):
                for i, xt in enumerate(xts):
                    nc.tensor.matmul(
                        pms[i],
                        w_s[:, ko, j * P:(j + 1) * P],
                        xt[:, ko, :],
                        start=(ko == 0),
                        stop=(ko == KO - 1),
                    )
            for i in range(len(xts)):
                nc.scalar.activation(
                    dsts[lbl][i][:, j, :], pms[i],
                    mybir.ActivationFunctionType.Identity,
                    bias=b_s[:, j:j + 1], scale=scl,
                )

        stages = []
        for lbl, w_s, b_s, scl in (("q", wq_s, bq16_s, SCALE), ("k", wk_s, bk_s, 1.0)):
            for j in range(DJ):
                stages.append(lambda lbl=lbl, w_s=w_s, b_s=b_s, scl=scl, j=j:
                              group(lbl, w_s, b_s, scl, j))
        return (dsts["q"], dsts["k"]), stages

    def load_and_transpose(b):
        x_sb = x_pool.tile([P, NCH, DM], F32R, name="x_sb", tag="x_sb")
        xr = x_d[b].rearrange("(c p) m -> p c m", p=P).bitcast(F32R)
        if split_x:
            for c in range(NCH):
                # alternate the two HWDGE queues (SP + ACT) so two HBM
                # streams run in parallel at each pair's load front
                eng = nc.scalar if (xq_split and c % 2) else nc.sync
                eng.dma_start(x_sb[:, c, :], xr[:, c, :])
        else:
            nc.sync.dma_start(x_sb, xr)
        xt = xt_pool.tile([P, KO, T], F32R, name="xt", tag="xt")
        # xT layout [dm_inner, ko, tok]; 4 transposes share one PSUM bank and
        # drain with a single wide DVE copy (amortizes the fixed PSUM access)
        for ko in range(KO):
            pt = ps_tr.tile([P, NCH, P], F32R, name="pt", tag="pt")
            for c in range(NCH):
                nc.tensor.transpose(
                    pt[:, c, :], x_sb[:, c, ko * P:(ko + 1) * P], identr
                )
            if alt_drain and ko % 2:
                # alternate drain engines so two PSUM banks drain in parallel
                # and the PE never waits for a free transpose bank
                nc.scalar.copy(xt[:, ko, :], pt)
            else:
                nc.vector.tensor_copy(xt[:, ko, :], pt)
        return xt

    def qk_projections(xts):
        # paired batches: one weight chunk (stationary) feeds both batches'
        # moving operands back-to-back -> one LDWEIGHTS per two matmuls
        out = []
        for lbl, w_s, b_s, scl in (("q", wq_s, bq16_s, SCALE), ("k", wk_s, bk_s, 1.0)):
            dsts = [
                qkv_pool.tile([P, DJ, T], F32R, name="qkt", tag=f"qkt{i}{lbl}")
                for i in range(len(xts))
            ]
            for j in range(DJ):
                pms = [ps_mm.tile([P, T], F32, name="pm", tag="pm") for _ in xts]
                for ko in range(KO):
                    for i, xt in enumerate(xts):
                        nc.tensor.matmul(
                            pms[i],
                            w_s[:, ko, j * P:(j + 1) * P],
                            xt[:, ko, :],
                            start=(ko == 0),
                            stop=(ko == KO - 1),
                        )
                for i in range(len(xts)):
                    # copy-back on ACT: dst = psum*scl + bias (q scaled 1/16)
                    nc.scalar.activation(
                        dsts[i][:, j, :], pms[i],
                        mybir.ActivationFunctionType.Identity,
                        bias=b_s[:, j:j + 1], scale=scl,
                    )
            out.append(dsts)
        return out  # [[qt_b0, qt_b1], [kt_b0, kt_b1]]

    def attention_stages(b, xt, qt, kt):
        """Returns the list of schedulable emit-closures for one batch:
        4 v-projection chunks + S/T/V softmax-attention stages per chunk."""
        v_sb = qkv_pool.tile([P, NCH, D], F32R, name="v_sb", tag=f"v_sb{b % 2}")
        stash_s = {}
        stash_t = {}

        def v_chunk(c):
            pv = ps_av.tile([P, D], F32, name="pv", tag="pav")
            for ko in range(KO):
                nc.tensor.matmul(
                    pv,
                    xt[:, ko, c * P:(c + 1) * P],
                    wv_s[:, ko, :],
                    start=(ko == 0),
                    stop=(ko == KO - 1),
                )
            nc.scalar.copy(v_sb[:, c, :], pv)

        # experimental (vt_proj): Wv-stationary projection at N=512 --
        # 16 MMs instead of 32 with reusable weight loads -- then PE-transpose
        # vT back to the natural [token, d] layout the AV matmul needs
        vt_sb = qkv_pool.tile([P, DJ, T], F32R, name="vt_sb",
                              tag=f"vt{b % 2}") if vt_proj else None

        def vt_mm(j):
            pm = ps_mm.tile([P, T], F32, name="pm", tag="pm")
            for ko in range(KO):
                nc.tensor.matmul(
                    pm,
                    wv_s[:, ko, j * P:(j + 1) * P],
                    xt[:, ko, :],
                    start=(ko == 0),
                    stop=(ko == KO - 1),
                )
            nc.scalar.copy(vt_sb[:, j, :], pm)

        def vt_tr(j):
            pt = ps_tr.tile([P, NCH, P], F32R, name="pt", tag="pt")
            for c in range(NCH):
                nc.tensor.transpose(
                    pt[:, c, :], vt_sb[:, j, c * P:(c + 1) * P], identr
                )
            nc.vector.tensor_copy(v_sb[:, :, j * P:(j + 1) * P], pt)

        def stage_s(c):
            L = (c + 1) * P  # causal: keys [0, L)
            # pad the c=0 matmul to 256 keys: fp32r runs 4x slower below a
            # 256-wide moving operand; the pad block is fully masked
            Lm = max(L, 2 * P) if c0pad else L
            ps = ps_s.tile([P, T], F32, name="ps", tag="ps")
            for j in range(DJ):
                nc.tensor.matmul(
                    ps[:, :Lm],
                    qt[:, j, c * P:(c + 1) * P],
                    kt[:, j, :Lm],
                    start=(j == 0),
                    stop=(j == DJ - 1),
                )
            # additive causal mask on the diagonal block
            nc.vector.tensor_add(ps[:, c * P:L], ps[:, c * P:L], cmask)
            if Lm > L:
                nc.vector.tensor_add(ps[:, L:Lm], ps[:, L:Lm], cfull)
            # scores are O(few): softmax without max-subtraction is safe, and
            # the Exp emits the row-sum in the same pass
            w_sb = w_pool.tile([P, T], F32R, name="w_sb", tag="w_sb")
            l_sb = stat_pool.tile([P, 1], F32, name="l_sb", tag="l_sb")
            nc.scalar.activation(
                w_sb[:, :Lm], ps[:, :Lm], mybir.ActivationFunctionType.Exp,
                scale=1.0, accum_out=l_sb,
            )
            linv = stat_pool.tile([P, 1], F32, name="linv", tag="linv")
            nc.vector.reciprocal(linv, l_sb)
            stash_s[c] = (w_sb, linv)

        def stage_t(c):
            w_sb, linv = stash_s.pop(c)
            wt = wt_pool.tile([P, NCH, P], F32R, name="wt", tag="wt")
            pt2 = ps_tr.tile([P, NCH, P], F32R, name="pt2", tag="pt")
            for s in range(c + 1):
                nc.tensor.transpose(pt2[:, s, :], w_sb[:, s * P:(s + 1) * P], identr)
            if alt_drain and c % 2:
                nc.scalar.copy(wt[:, :c + 1, :], pt2[:, :c + 1, :])
            else:
                nc.vector.tensor_copy(wt[:, :c + 1, :], pt2[:, :c + 1, :])
            stash_t[c] = (wt, linv)

        def stage_v(c):
            wt, linv = stash_t.pop(c)
            po = ps_av.tile([P, D], F32, name="po", tag="pav")
            for s in range(c + 1):
                nc.tensor.matmul(
                    po, wt[:, s, :], v_sb[:, s, :],
                    start=(s == 0), stop=(s == c),
                )
            # out = (w @ v_nobias) / l ... + bv (bias passes through softmax)
            ot = o_pool.tile([P, D], F32, name="ot", tag="ot")
            nc.scalar.activation(
                ot, po, mybir.ActivationFunctionType.Copy, scale=linv,
            )
            oc = o_pool.tile([P, D], F32, name="oc", tag="oc")
            nc.gpsimd.tensor_add(oc, ot, bv_s)
            if gp_store:
                nc.gpsimd.dma_start(out_d[b, c * P:(c + 1) * P, :], oc)
            else:
                nc.sync.dma_start(out_d[b, c * P:(c + 1) * P, :], oc)

        if vt_proj:
            stages = [("vp", vt_mm, 0), ("vp", vt_mm, 1),
                      ("vp", vt_tr, 0), ("vp", vt_tr, 1)]
        else:
            stages = [("vp", v_chunk, c) for c in range(NCH)]
        if stv:
            order = [("s", 0), ("s", 1), ("t", 0), ("s", 2), ("t", 1), ("v", 0),
                     ("s", 3), ("t", 2), ("v", 1), ("t", 3), ("v", 2), ("v", 3)]
        else:
            order = [(k, c) for c in range(NCH) for k in ("s", "t", "v")]
        fmap = {"s": stage_s, "t": stage_t, "v": stage_v}
        stages += [(k, fmap[k], c) for k, c in order]
        return stages

    if xpair and pair_qk and dual:
        # pair-level software pipeline: pair p's loads/transposes/projections
        # are emitted riffled with pair p-1's attention stages, so each
        # phase's PE stalls are filled by the other's independent matmuls
        pending = None
        for pi, b0 in enumerate(range(0, BPC, 2)):
            xt0, ls0 = load_stages(b0)
            xt1, ls1 = load_stages(b0 + 1)
            prep = [s for pair in zip(ls0, ls1) for s in pair]
            if pi == 0:
                prep.insert(2, load_consts)
            (qts, kts), qs = qk_proj_stages([xt0, xt1])
            prep += qs
            if pending is None:
                for s in prep:
                    s()
            else:
                n = max(len(pending), len(prep))
                for i in range(n):
                    if i < len(pending):
                        _k, fn, c = pending[i]
                        fn(c)
                    if i < len(prep):
                        prep[i]()
            a0 = attention_stages(b0, xt0, qts[0], kts[0])
            a1 = attention_stages(b0 + 1, xt1, qts[1], kts[1])
            pending = [s for pair in zip(a0, a1) for s in pair]
        for _k, fn, c in pending:
            fn(c)
        return

    consts_loaded = [False]
    step = 2 if pair_qk else 1
    for b0 in range(0, BPC, step):
        xts = [load_and_transpose(b0 + i) for i in range(step)]
        if not consts_loaded[0]:
            load_consts()
            consts_loaded[0] = True
        (qts, kts) = qk_projections(xts)
        lists = [
            attention_stages(b0 + i, xts[i], qts[i], kts[i])
            for i in range(step)
        ]
        if step == 2 and not dual:
            for lst in lists:
                for _k, fn, c in lst:
                    fn(c)
        elif step == 2:
            # strict alternation of the two batches' pipelines: each batch's
            # stages provide PE fill for the other's softmax latencies
            a, bl = lists
            merged = []
            for sa, sb in zip(a, bl):
                merged.append(sa)
                merged.append(sb)
            for _k, fn, c in merged:
                fn(c)
        else:
            for _k, fn, c in lists[0]:
                fn(c)



def build_program(reps=1, hints=True, **flags):
    """Build the single-core Bass program (same program runs on all 8 cores).

    reps > 1 wraps the whole body in a hardware loop (same work each
    iteration) -- used only for device-time measurement."""
    nc = bacc.Bacc("TRN2", target_bir_lowering=False, debug=False)
    x_d = nc.dram_tensor("x", [BPC, T, DM], F32, kind="ExternalInput").ap()
    wq_d = nc.dram_tensor("wq", [DM, D], F32, kind="ExternalInput").ap()
    bq_d = nc.dram_tensor("bq", [D], F32, kind="ExternalInput").ap()
    wk_d = nc.dram_tensor("wk", [DM, D], F32, kind="ExternalInput").ap()
    bk_d = nc.dram_tensor("bk", [D], F32, kind="ExternalInput").ap()
    wv_d = nc.dram_tensor("wv", [DM, D], F32, kind="ExternalInput").ap()
    bv_d = nc.dram_tensor("bv", [D], F32, kind="ExternalInput").ap()
    out_d = nc.dram_tensor("out", [BPC, T, D], F32, kind="ExternalOutput").ap()

    from contextlib import ExitStack

    with tile.TileContext(nc) as tc, ExitStack() as ctx:
        emit_core_program(
            ctx, nc, tc, (x_d, wq_d, bq_d, wk_d, bk_d, wv_d, bv_d, out_d),
            reps=reps, hints=hints, **flags,
        )
    nc.compile()
    return nc


_NC_CACHE = None


def _get_program():
    global _NC_CACHE
    if _NC_CACHE is None:
        _NC_CACHE = build_program()
    return _NC_CACHE


def make_in_maps(inputs):
    x = np.ascontiguousarray(np.asarray(inputs["x"], dtype=np.float32))
    shared = {
        "wq": np.ascontiguousarray(np.asarray(inputs["Wq"], np.float32)),
        "bq": np.ascontiguousarray(np.asarray(inputs["bq"], np.float32)),
        "wk": np.ascontiguousarray(np.asarray(inputs["Wk"], np.float32)),
        "bk": np.ascontiguousarray(np.asarray(inputs["bk"], np.float32)),
        "wv": np.ascontiguousarray(np.asarray(inputs["Wv"], np.float32)),
        "bv": np.ascontiguousarray(np.asarray(inputs["bv"], np.float32)),
    }
    return [
        {"x": x[i * BPC:(i + 1) * BPC], **shared} for i in range(NCORES)
    ]


def kernel(**inputs) -> np.ndarray:
    nc = _get_program()
    in_maps = make_in_maps(inputs)
    res = run_bass_kernel_spmd(nc, in_maps, core_ids=list(range(NCORES)))
    return np.concatenate([m["out"] for m in res.results], axis=0)

